# revision 23
# baseline (speedup 1.0000x reference)
"""Multi-head attention (qkv pointwise-conv projection + softmax attention)
on 8 Trainium2 NeuronCores.

Problem shapes (hardcoded):
    x:     [B=4, D=512, L=2048] f32
    w_qkv: [3*D=1536, D=512]    f32
    out:   [B, D, L]            f32

Sharding: 2 cores per batch element; each core owns 4 of the 8 heads
(tensor-parallel on the qkv output channels). Core c -> batch c//2,
head group c%2 (heads 4*(c%2) .. 4*(c%2)+3).

Per-core kernel (all in bf16 compute, f32 accumulate):
    Q/K proj:  q[o,l] = sum_d w[o,d] x[d,l]   (layout [head_dim, L])
    V proj  :  vT[l,o]                          (layout [L, head_dim])
               vT stored per head with a fused ones-column -> attn@[v|1]
               yields both the weighted values and the softmax denominator.
    scores  :  St[j,i] = sum_d k[d,j] q[d,i]  (two heads packed in the
               128-row PE array via row tiling: head0 partitions 0-63,
               head1 partitions 64-127)
    softmax :  exp on ScalarE (scale folded into the activation), no max
               subtraction (scores are O(1) by construction)
    attn@v  :  O[d(+den),i] accumulated over j blocks in PSUM
    norm    :  O[d,i] * broadcast(1/den[i])  (broadcast via K=1 matmul)

ScalarE is the bottleneck (128 exps x ~1.11us = 142us of 180us): the
schedule aims to keep the exp stream gapless. Startup: input DMAs go out
on three parallel DGE queues (SP + Act HWDGE, Pool SWDGE) with x chunk 0
split in halves and w_qkv split in per-(pair,q|k) sections so the first
scores land ~4us earlier. Fillers: pair-1 projections run inside pair-1
blocks (each block has ~4us of PE slack vs the exp cadence), k0 column
groups are front-loaded in block 0, vt is just-in-time. Tail: the final
block normalizes via ScalarE den-copies (idle after the last exp),
GpSimd does one of the two multiplies, and the two output DMAs use
separate queues.
"""

import os
import numpy as np

B, D, L, H = 4, 512, 2048, 8
HD = D // H  # 64
N_CORES = 8
SCALE = float(D) ** -0.5

# module-level knobs for test.py; harness uses defaults
TRACE = False
LAST_RESULTS = None

_COMPILED = {}


def _build_nc():
    from contextlib import ExitStack

    import concourse.bass as bass
    import concourse.mybir as mybir
    import concourse.tile as tile
    from concourse.bacc import Bacc

    F32 = mybir.dt.float32
    BF16 = mybir.dt.bfloat16
    Exp = mybir.ActivationFunctionType.Exp

    # Bacc (not plain Bass): its finalize() runs the legalization passes that
    # split multi-wait matmuls (walrus MM struct supports only 1 sync wait).
    nc = Bacc("TRN2", target_bir_lowering=False, debug=False)
    # host pre-permuted layouts -> fully contiguous DMA descriptors
    # x: [p, lc, dc, l'] where d = dc*128+p, l = lc*512+l'
    x_d = nc.dram_tensor("x", [128, 4, 4, 512], BF16, kind="ExternalInput")
    # wT split in 4 sections s = 2*pair + (0 q | 1 k): [s, p, dc, o]
    wqk_d = nc.dram_tensor("wqkT", [4, 128, 4, 128], BF16, kind="ExternalInput")
    wv_d = nc.dram_tensor("wvT", [128, 4, 256], BF16, kind="ExternalInput")
    out_d = nc.dram_tensor("out", [256, L], F32, kind="ExternalOutput")

    NJB = L // 128  # 16 key blocks
    NIC = L // 512  # 4 query chunks

    with ExitStack() as ctx:
        tc = ctx.enter_context(tile.TileContext(nc))
        const = ctx.enter_context(tc.tile_pool(name="const", bufs=1))
        qkp = ctx.enter_context(tc.tile_pool(name="qkp", bufs=1))
        vtp = ctx.enter_context(tc.tile_pool(name="vtp", bufs=1))
        sx = ctx.enter_context(tc.tile_pool(name="sx", bufs=6))
        sx2 = ctx.enter_context(tc.tile_pool(name="sx2", bufs=3))
        nrm = ctx.enter_context(tc.tile_pool(name="nrm", bufs=4))
        outp = ctx.enter_context(tc.tile_pool(name="outp", bufs=4))
        drp = ctx.enter_context(tc.tile_pool(name="drp", bufs=4, space="DRAM"))
        # PSUM phase A (pair-0 blocks + block (1,0), which carry projection
        # fillers): 2x[128,1024] st ring + 4x[128,512] shared o/proj ring.
        # Released after block (1,0) and replaced by a single [128,3072] tile
        # (manual 3-slot ring, subtile deps) + 2x[128,512] o ring so the last
        # three (filler-light) blocks can run 2048-wide exps.
        ps_st = tc.alloc_tile_pool(name="ps_st", bufs=2, space="PSUM")
        ps_o = tc.alloc_tile_pool(name="ps_o", bufs=4, space="PSUM")

        # ---- PE warmup + load inputs ----
        # a few matmuls on zeros keep the PE busy while the first input DMAs
        # land, so the HAM clock gate starts opening (1.2 -> 2.4 GHz).
        scr_sb = const.tile([128, 512], BF16, tag="scr")
        nc.vector.memset(scr_sb[:], 0.0)
        warm_ps = ps_st.tile([128, 1024], F32, tag="st", name="warm")
        for _ in range(6):
            nc.tensor.matmul(warm_ps[:, 0:512], scr_sb[:, 0:128], scr_sb[:])

        wqk_sb = const.tile([128, 4, 4, 128], BF16, tag="wqk")
        wv_sb = const.tile([128, 4, 256], BF16, tag="wv")
        x_sb = const.tile([128, 4, 4, 512], BF16, tag="x")
        # Each HWDGE queue sustains only ~200GB/s but drains FIFO, so: split
        # the critical first-projection set (wqk sections 0-1 + x chunk 0)
        # across BOTH rings so it lands in ~1.2us per ring, with the bulk
        # chunks queued behind it in deadline order. Config instructions also
        # serialize per-ring (~0.65us each) — another reason to use two.
        nc.sync.dma_start(out=wqk_sb[:, 0, :, :], in_=wqk_d[0])
        nc.scalar.dma_start(out=wqk_sb[:, 1, :, :], in_=wqk_d[1])
        nc.sync.dma_start(out=x_sb[:, 0, 0:2, :], in_=x_d[:, 0, 0:2, :])
        nc.scalar.dma_start(out=x_sb[:, 0, 2:4, :], in_=x_d[:, 0, 2:4, :])
        nc.sync.dma_start(out=x_sb[:, 1, :, :], in_=x_d[:, 1, :, :])
        nc.scalar.dma_start(out=wv_sb[:], in_=wv_d[:])
        nc.sync.dma_start(out=x_sb[:, 3, :, :], in_=x_d[:, 3, :, :])
        nc.scalar.dma_start(out=x_sb[:, 2, :, :], in_=x_d[:, 2, :, :])
        nc.scalar.dma_start(out=wqk_sb[:, 2, :, :], in_=wqk_d[2])
        nc.scalar.dma_start(out=wqk_sb[:, 3, :, :], in_=wqk_d[3])
        ones_sb = const.tile([1, 64], F32, tag="ones")
        nc.vector.memset(ones_sb[:], 1.0)

        q_sb = [qkp.tile([128, L], BF16, tag=f"q{p}", name=f"q{p}") for p in range(2)]
        k_sb = [qkp.tile([128, L], BF16, tag=f"k{p}", name=f"k{p}") for p in range(2)]
        vt_sb = [vtp.tile([128, 4, 65], BF16, tag=f"vt{jb}", name=f"vt{jb}") for jb in range(NJB)]

        # Projection groups run in 1-bank [128,512] PSUM tiles from the shared
        # "o" pool so they never contend with the exp-feeding st pipeline.
        def g_qk(p, qk, lc):
            # one 512-wide column group of the Q (qk=0) or K (qk=1)
            # projection for head-pair p
            def f():
                dst = q_sb[p] if qk == 0 else k_sb[p]
                ps = ps_o.tile([128, 512], F32, tag="o", name="projg")
                for dc in range(4):
                    nc.tensor.matmul(
                        ps[:],
                        wqk_sb[:, 2 * p + qk, dc, :],
                        x_sb[:, lc, dc, :],
                        start=(dc == 0),
                        stop=(dc == 3),
                    )
                nc.vector.tensor_copy(dst[:, lc * 512 : (lc + 1) * 512], ps[:])

            return f

        def g_qk_split(p, qk, lc, dcs):
            # g_qk sub-split into per-iteration pieces (each <=2 matmuls,
            # ~426ns) so filler load never spikes a single exp interval.
            # Returns a list of callables; schedule them on consecutive jbs.
            state = {}

            def part(dc_list):
                def f():
                    if 0 in dc_list:
                        state["ps"] = ps_o.tile(
                            [128, 512], F32, tag="o", name="projg"
                        )
                    for dc in dc_list:
                        nc.tensor.matmul(
                            state["ps"][:],
                            wqk_sb[:, 2 * p + qk, dc, :],
                            x_sb[:, lc, dc, :],
                            start=(dc == 0),
                            stop=(dc == 3),
                        )
                    if 3 in dc_list:
                        dst = q_sb[p] if qk == 0 else k_sb[p]
                        nc.vector.tensor_copy(
                            dst[:, lc * 512 : (lc + 1) * 512], state["ps"][:]
                        )

                return f

            return [part(dcl) for dcl in dcs]

        def g_vt(jb):
            def f():
                nc.vector.memset(vt_sb[jb][:, :, 64:65], 1.0)
                ps = ps_o.tile([128, 512], F32, tag="o", name="projv")
                for dc in range(4):
                    nc.tensor.matmul(
                        ps[:, 0:256],
                        x_sb[:, jb // 4, dc, (jb % 4) * 128 : (jb % 4 + 1) * 128],
                        wv_sb[:, dc, :],
                        start=(dc == 0),
                        stop=(dc == 3),
                    )
                nc.vector.tensor_copy(
                    vt_sb[jb][:, :, 0:64],
                    ps[:, 0:256].rearrange("par (h e) -> par h e", e=64),
                )

            return f

        def attn_block(p, ic, fillers=(), last=False):
            # scores+softmax+attn@v for head pair p, query chunk ic (512 wide)
            # fillers: {jb: [callables]} — projection groups interleaved into
            # the loop to fill PE slack without starving ScalarE
            # last: tail-optimized normalization (ScalarE den copies, GpSimd
            # mul, parallel out-DMA queues, PE broadcast)
            fillers = dict(fillers)
            i0 = ic * 512

            def st_mms(jb):
                # St[j, i] for both heads of the pair, row-packed in the PE
                st = ps_st.tile([128, 1024], F32, tag="st")
                for hp in range(2):
                    nc.tensor.matmul(
                        st[:, hp * 512 : (hp + 1) * 512],
                        k_sb[p][hp * 64 : (hp + 1) * 64, jb * 128 : (jb + 1) * 128],
                        q_sb[p][hp * 64 : (hp + 1) * 64, i0 : i0 + 512],
                        start=True,
                        stop=True,
                    )
                return st

            o_ps = [ps_o.tile([65, 512], F32, tag="o", name="o_acc") for _ in range(2)]
            st_cur = st_mms(0)
            for jb in range(NJB):
                se = sx.tile([128, 1024], BF16, tag="se")
                nc.scalar.activation(se[:], st_cur[:], Exp, scale=SCALE)
                if jb + 1 < NJB:
                    st_cur = st_mms(jb + 1)
                for f in fillers.get(jb, ()):
                    f()
                for hp in range(2):
                    nc.tensor.matmul(
                        o_ps[hp][:],
                        vt_sb[jb][:, 2 * p + hp, :],
                        se[:, hp * 512 : (hp + 1) * 512],
                        start=(jb == 0),
                        stop=(jb == NJB - 1),
                    )

            if not last:
                # normalize and write out. The o_ps ring slots gate the NEXT
                # block's projection fillers, so the PSUM accumulator must be
                # released ASAP: ONE [65,512] DVE copy (same cost as copying
                # just the den row — DVE time is free-dim-bound) moves o+den
                # to SBUF, then the whole chain (1/den on DVE, broadcast via
                # DRAM bounce, multiply) runs off SBUF without touching PSUM.
                for hp in range(2):
                    hh = 2 * p + hp
                    den_sb = nrm.tile([1, 512], F32, tag="den")
                    nc.vector.tensor_copy(den_sb[:], o_ps[hp][64:65, :])
                    oc = nrm.tile([64, 512], F32, tag="oc")
                    nc.vector.tensor_copy(oc[:], o_ps[hp][0:64, :])
                    recip = nrm.tile([1, 512], F32, tag="recip")
                    # NB: approx-recip reads garbage from PSUM on HW; SBUF in
                    # only, and DVE lanes are partition-aligned: in/out must
                    # start at the same partition (hence the den copy to p0).
                    nc.vector.reciprocal_approx_fast(out=recip[:], in_=den_sb[:])
                    rbc = nrm.tile([64, 512], F32, tag="rbc")
                    dbounce = drp.tile([1, 512], F32, tag="db", name="db")
                    nc.sync.dma_start(out=dbounce[:], in_=recip[:])
                    nc.sync.dma_start(
                        out=rbc[:],
                        in_=bass.AP(
                            tensor=dbounce.tensor,
                            offset=dbounce.offset,
                            ap=[[0, 64], [1, 512]],
                        ),
                    )
                    ot = outp.tile([64, 512], F32, tag="ot")
                    nc.vector.tensor_mul(ot[:], oc[0:64, :], rbc[:])
                    nc.sync.dma_start(
                        out=out_d[hh * 64 : (hh + 1) * 64, i0 : i0 + 512], in_=ot[:]
                    )
                return

            # tail-optimized final normalization. Hop count is what matters:
            # den-copy + reciprocal back-to-back on DVE (no cross-engine sem
            # between them), broadcast matmul on a pre-warmed PE (six dummy
            # matmuls run during the last exp so the HAM clock stays up),
            # broadcast read-back on the idle ScalarE, muls on DVE, and the
            # two out-DMAs on separate queues.
            den_sb = [nrm.tile([1, 512], F32, tag="den", name=f"den{hp}") for hp in range(2)]
            oc = [nrm.tile([64, 512], F32, tag="oc", name=f"oc{hp}") for hp in range(2)]
            recip = [nrm.tile([1, 512], F32, tag="recip", name=f"recip{hp}") for hp in range(2)]
            rbc = [nrm.tile([64, 512], F32, tag="rbc", name=f"rbc{hp}") for hp in range(2)]
            bc_ps = [ps_st.tile([128, 1024], F32, tag="st", name=f"bc{hp}") for hp in range(2)]
            ot = [outp.tile([64, 512], F32, tag="ot", name=f"ot{hp}") for hp in range(2)]
            for _ in range(4):
                nc.tensor.matmul(
                    bc_ps[0][64:128, 0:512], scr_sb[:, 0:64], scr_sb[:],
                    start=True, stop=True,
                )
            for hp in range(2):
                nc.vector.tensor_copy(den_sb[hp][:], o_ps[hp][64:65, :])
                nc.vector.reciprocal_approx_fast(out=recip[hp][:], in_=den_sb[hp][:])
                nc.scalar.copy(oc[hp][:], o_ps[hp][0:64, :])
            for _ in range(3):
                nc.tensor.matmul(
                    bc_ps[0][64:128, 0:512], scr_sb[:, 0:64], scr_sb[:],
                    start=True, stop=True,
                )
            for hp in range(2):
                nc.tensor.matmul(
                    bc_ps[hp][0:64, 0:512], ones_sb[:], recip[hp][:],
                    start=True, stop=True,
                )
                nc.scalar.copy(rbc[hp][:], bc_ps[hp][0:64, 0:512])
            # muls are all-SBUF (oc x rbc), so GpSimd can take one in parallel
            nc.gpsimd.tensor_mul(ot[0][:], oc[0][:], rbc[0][:])
            nc.vector.tensor_mul(ot[1][:], oc[1][:], rbc[1][:])
            hh0 = 2 * p
            nc.sync.dma_start(
                out=out_d[hh0 * 64 : (hh0 + 1) * 64, i0 : i0 + 512], in_=ot[0][:]
            )
            nc.scalar.dma_start(
                out=out_d[(hh0 + 1) * 64 : (hh0 + 2) * 64, i0 : i0 + 512], in_=ot[1][:]
            )

        def attn_block_big(p, ic, stbig, ps_o2, fillers=(), last=False):
            # 2048-wide-exp variant for the filler-light final blocks: scores
            # for jb PAIRS land in a manual 3-slot ring inside one [128,3072]
            # PSUM tile; each ACTIVATE covers two key blocks (halving the
            # 352-cycle per-instruction overhead). When the pair's slots are
            # (2,0) the ascending-offset AP reads them swapped — the av
            # matmuls then just read the opposite se halves.
            fillers = dict(fillers)
            i0 = ic * 512
            SLOT = [0, 1024, 2048]

            def st_mm(jb):
                sl = SLOT[jb % 3]
                for hp in range(2):
                    nc.tensor.matmul(
                        stbig[:, sl + hp * 512 : sl + (hp + 1) * 512],
                        k_sb[p][hp * 64 : (hp + 1) * 64, jb * 128 : (jb + 1) * 128],
                        q_sb[p][hp * 64 : (hp + 1) * 64, i0 : i0 + 512],
                        start=True,
                        stop=True,
                    )

            o_ps = [ps_o2.tile([65, 512], F32, tag="o", name="o_acc") for _ in range(2)]
            st_mm(0)
            st_mm(1)
            for pk in range(8):
                jb0, jb1 = 2 * pk, 2 * pk + 1
                s0, s1 = SLOT[jb0 % 3], SLOT[jb1 % 3]
                lo, hi = min(s0, s1), max(s0, s1)
                se = sx2.tile([128, 2048], BF16, tag="se2")
                big3 = stbig.rearrange("par (c n) -> par c n", n=1024)
                step = (hi - lo) // 1024
                in_ap = big3[:, lo // 1024 : hi // 1024 + 1 : step, :]
                nc.scalar.activation(
                    se.rearrange("par (two n) -> par two n", two=2),
                    in_ap,
                    Exp,
                    scale=SCALE,
                )
                half = {jb0: (0 if s0 == lo else 1), jb1: (0 if s1 == lo else 1)}
                if jb1 + 1 < NJB:
                    st_mm(jb1 + 1)
                if jb1 + 2 < NJB:
                    st_mm(jb1 + 2)
                for f in fillers.get(pk, ()):
                    f()
                for jb in (jb0, jb1):
                    h0 = half[jb] * 1024
                    for hp in range(2):
                        nc.tensor.matmul(
                            o_ps[hp][:],
                            vt_sb[jb][:, 2 * p + hp, :],
                            se[:, h0 + hp * 512 : h0 + (hp + 1) * 512],
                            start=(jb == 0),
                            stop=(jb == NJB - 1),
                        )

            if not last:
                for hp in range(2):
                    hh = 2 * p + hp
                    den_sb = nrm.tile([1, 512], F32, tag="den")
                    nc.vector.tensor_copy(den_sb[:], o_ps[hp][64:65, :])
                    oc = nrm.tile([64, 512], F32, tag="oc")
                    nc.vector.tensor_copy(oc[:], o_ps[hp][0:64, :])
                    recip = nrm.tile([1, 512], F32, tag="recip")
                    nc.vector.reciprocal_approx_fast(out=recip[:], in_=den_sb[:])
                    rbc = nrm.tile([64, 512], F32, tag="rbc")
                    dbounce = drp.tile([1, 512], F32, tag="db", name="db")
                    nc.sync.dma_start(out=dbounce[:], in_=recip[:])
                    nc.sync.dma_start(
                        out=rbc[:],
                        in_=bass.AP(
                            tensor=dbounce.tensor,
                            offset=dbounce.offset,
                            ap=[[0, 64], [1, 512]],
                        ),
                    )
                    ot = outp.tile([64, 512], F32, tag="ot")
                    nc.vector.tensor_mul(ot[:], oc[:], rbc[:])
                    nc.sync.dma_start(
                        out=out_d[hh * 64 : (hh + 1) * 64, i0 : i0 + 512], in_=ot[:]
                    )
                return

            # tail: slot 1 (cols 1024:2048) was last read two exps ago — use
            # it for the PE warm-up + broadcast matmuls.
            den_sb = [nrm.tile([1, 512], F32, tag="den", name=f"den{hp}") for hp in range(2)]
            oc = [nrm.tile([64, 512], F32, tag="oc", name=f"oc{hp}") for hp in range(2)]
            recip = [nrm.tile([1, 512], F32, tag="recip", name=f"recip{hp}") for hp in range(2)]
            rbc = [nrm.tile([64, 512], F32, tag="rbc", name=f"rbc{hp}") for hp in range(2)]
            ot = [outp.tile([64, 512], F32, tag="ot", name=f"ot{hp}") for hp in range(2)]
            for _ in range(4):
                nc.tensor.matmul(
                    stbig[64:128, 1024:1536], scr_sb[:, 0:64], scr_sb[:],
                    start=True, stop=True,
                )
            for hp in range(2):
                nc.vector.tensor_copy(den_sb[hp][:], o_ps[hp][64:65, :])
                nc.vector.reciprocal_approx_fast(out=recip[hp][:], in_=den_sb[hp][:])
                nc.scalar.copy(oc[hp][:], o_ps[hp][0:64, :])
            for _ in range(3):
                nc.tensor.matmul(
                    stbig[64:128, 1024:1536], scr_sb[:, 0:64], scr_sb[:],
                    start=True, stop=True,
                )
            for hp in range(2):
                nc.tensor.matmul(
                    stbig[0:64, 1024 + hp * 512 : 1536 + hp * 512],
                    ones_sb[:], recip[hp][:],
                    start=True, stop=True,
                )
                nc.scalar.copy(rbc[hp][:], stbig[0:64, 1024 + hp * 512 : 1536 + hp * 512])
            nc.gpsimd.tensor_mul(ot[0][:], oc[0][:], rbc[0][:])
            nc.vector.tensor_mul(ot[1][:], oc[1][:], rbc[1][:])
            hh0 = 2 * p
            nc.sync.dma_start(
                out=out_d[hh0 * 64 : (hh0 + 1) * 64, i0 : i0 + 512], in_=ot[0][:]
            )
            nc.scalar.dma_start(
                out=out_d[(hh0 + 1) * 64 : (hh0 + 2) * 64, i0 : i0 + 512], in_=ot[1][:]
            )

        # prologue: just enough projection for the first scores (q0/k0 column
        # group 0); everything else is interleaved as fillers.
        # Constraints: vt[j] before av(j) of block (0,0); k[p] group m fully
        # projected+cast before st(4m) is EMITTED (iter 4m-1) of the first
        # block using pair p; q[p] group lc before block (p,lc). Projection
        # groups are sub-split so no single exp interval carries more than
        # ~0.43us of filler on top of st+av(+vt).
        def sched(*entries):
            m = {}
            for start, parts in entries:
                for i, part in enumerate(parts):
                    m.setdefault(start + i, []).append(part)
            return m

        SPLIT_211 = [[0, 1], [2], [3]]
        SPLIT_1111 = [[0], [1], [2], [3]]

        g_qk(0, 0, 0)()
        g_qk(0, 1, 0)()
        attn_block(0, 0, sched(
            *[(jb, [g_vt(jb)]) for jb in range(NJB)],
            (0, g_qk_split(0, 1, 1, SPLIT_211)),
            (3, g_qk_split(0, 1, 2, SPLIT_211)),
            (7, g_qk_split(0, 1, 3, SPLIT_1111)),
            (11, g_qk_split(0, 0, 1, SPLIT_1111)),
        ))
        attn_block(0, 1, sched((2, g_qk_split(0, 0, 2, SPLIT_1111))))
        attn_block(0, 2, sched((2, g_qk_split(0, 0, 3, SPLIT_1111))))
        attn_block(0, 3, sched(
            (1, g_qk_split(1, 1, 0, SPLIT_1111)),
            (8, g_qk_split(1, 0, 0, SPLIT_1111)),
        ))
        # block (1,0) also absorbs the q1 lc2/lc3 projections (phase-A PSUM is
        # needed for proj scratch) so the final three blocks are filler-free.
        q1lc2 = g_qk_split(1, 0, 2, SPLIT_1111)
        q1lc3 = g_qk_split(1, 0, 3, SPLIT_1111)
        attn_block(1, 0, sched(
            (0, g_qk_split(1, 1, 1, SPLIT_211)),
            (3, g_qk_split(1, 1, 2, SPLIT_1111)),
            (7, g_qk_split(1, 1, 3, SPLIT_1111)),
            (11, g_qk_split(1, 0, 1, SPLIT_1111)),
            (1, [q1lc2[0]]), (2, [q1lc2[1]]), (4, [q1lc2[2]]), (5, [q1lc2[3]]),
            (6, [q1lc3[0]]), (8, [q1lc3[1]]), (9, [q1lc3[2]]), (12, [q1lc3[3]]),
        ))
        # PSUM phase B: one [128,3072] score tile (3-slot ring) + 2-slot o ring
        # (pools must release in LIFO order)
        ps_o.release()
        ps_st.release()
        ps_big = tc.alloc_tile_pool(name="ps_big", bufs=1, space="PSUM")
        stbig = ps_big.tile([128, 3072], F32, tag="big", name="stbig")
        ps_o2 = tc.alloc_tile_pool(name="ps_o2", bufs=2, space="PSUM")
        attn_block_big(1, 1, stbig, ps_o2)
        attn_block_big(1, 2, stbig, ps_o2)
        attn_block_big(1, 3, stbig, ps_o2, last=True)
        ps_o2.release()
        ps_big.release()

    nc.finalize()
    return nc


def _get_nc():
    if "nc" not in _COMPILED:
        _COMPILED["nc"] = _build_nc()
    return _COMPILED["nc"]


def _prep_inputs(x, w_qkv):
    """Per-core input maps (host-side sharding)."""
    import ml_dtypes

    bf16 = ml_dtypes.bfloat16
    in_maps = []
    for c in range(N_CORES):
        b, g = c // 2, c % 2
        # x[b] [512, 2048] -> [p, lc, dc, l'] so every DMA descriptor is a
        # 4KB contiguous run
        xb = np.ascontiguousarray(
            x[b].reshape(4, 128, 4, 512).transpose(1, 2, 0, 3)
        ).astype(bf16)
        # w rows for this head group, transposed then laid out in 4 sections
        # s = 2*pair + (0 q | 1 k): [s, p, dc, o]; v separate — all DMAs are
        # fully contiguous
        wq_rows = w_qkv[256 * g : 256 * (g + 1), :]  # [256, 512]
        wk_rows = w_qkv[512 + 256 * g : 512 + 256 * (g + 1), :]
        wv_rows = w_qkv[1024 + 256 * g : 1024 + 256 * (g + 1), :]
        # section s covers head pair p = s//2, q (s%2==0) or k: 128 columns of
        # wT = rows 128*p..128*(p+1) of the q|k block
        secs = []
        for p in range(2):
            for rows in (wq_rows, wk_rows):
                sec = rows[128 * p : 128 * (p + 1), :].T  # [512(d), 128(o)]
                secs.append(sec.reshape(4, 128, 128).transpose(1, 0, 2))
        wqkT = np.ascontiguousarray(np.stack(secs, axis=0)).astype(bf16)
        wvT = np.ascontiguousarray(
            wv_rows.T.reshape(4, 128, 256).transpose(1, 0, 2)
        ).astype(bf16)
        in_maps.append({"x": xb, "wqkT": wqkT, "wvT": wvT})
    return in_maps


def kernel(x, w_qkv):
    global LAST_RESULTS
    from concourse.bass_utils import run_bass_kernel_spmd

    nc = _get_nc()
    in_maps = _prep_inputs(np.asarray(x), np.asarray(w_qkv))
    res = run_bass_kernel_spmd(
        nc, in_maps, core_ids=list(range(N_CORES)), trace=TRACE
    )
    LAST_RESULTS = res
    out = np.empty((B, D, L), dtype=np.float32)
    for c in range(N_CORES):
        b, g = c // 2, c % 2
        out[b, 256 * g : 256 * (g + 1), :] = res.results[c]["out"]
    return out


# revision 25
# speedup vs baseline: 1.1346x; 1.1346x over previous
"""Multi-head attention (qkv pointwise-conv projection + softmax attention)
on 8 Trainium2 NeuronCores.

Problem shapes (hardcoded):
    x:     [B=4, D=512, L=2048] f32
    w_qkv: [3*D=1536, D=512]    f32
    out:   [B, D, L]            f32

Sharding: 2 cores per batch element; each core owns 4 of the 8 heads
(tensor-parallel on the qkv output channels). Core c -> batch c//2,
head group c%2 (heads 4*(c%2) .. 4*(c%2)+3).

Per-core kernel (all in bf16 compute, f32 accumulate):
    Q/K proj:  q[o,l] = sum_d w[o,d] x[d,l]   (layout [head_dim, L])
    V proj  :  vT[l,o]                          (layout [L, head_dim])
               vT stored per head with a fused ones-column -> attn@[v|1]
               yields both the weighted values and the softmax denominator.
    scores  :  St[j,i] = sum_d k[d,j] q[d,i]  (two heads packed in the
               128-row PE array via row tiling: head0 partitions 0-63,
               head1 partitions 64-127)
    softmax :  exp on ScalarE (scale folded into the activation), no max
               subtraction (scores are O(1) by construction)
    attn@v  :  O[d(+den),i] accumulated over j blocks in PSUM
    norm    :  O[d,i] * broadcast(1/den[i])  (broadcast via K=1 matmul)

ScalarE is the bottleneck (128 exps x ~1.11us = 142us of 180us): the
schedule aims to keep the exp stream gapless. Startup: input DMAs go out
on three parallel DGE queues (SP + Act HWDGE, Pool SWDGE) with x chunk 0
split in halves and w_qkv split in per-(pair,q|k) sections so the first
scores land ~4us earlier. Fillers: pair-1 projections run inside pair-1
blocks (each block has ~4us of PE slack vs the exp cadence), k0 column
groups are front-loaded in block 0, vt is just-in-time. Tail: the final
block normalizes via ScalarE den-copies (idle after the last exp),
GpSimd does one of the two multiplies, and the two output DMAs use
separate queues.
"""

import os
import numpy as np

B, D, L, H = 4, 512, 2048, 8
HD = D // H  # 64
N_CORES = 8
SCALE = float(D) ** -0.5

# module-level knobs for test.py; harness uses defaults
TRACE = False
LAST_RESULTS = None

_COMPILED = {}


def _build_nc():
    from contextlib import ExitStack

    import concourse.bass as bass
    import concourse.mybir as mybir
    import concourse.tile as tile
    from concourse.bacc import Bacc

    F32 = mybir.dt.float32
    BF16 = mybir.dt.bfloat16
    Exp = mybir.ActivationFunctionType.Exp

    # Bacc (not plain Bass): its finalize() runs the legalization passes that
    # split multi-wait matmuls (walrus MM struct supports only 1 sync wait).
    nc = Bacc("TRN2", target_bir_lowering=False, debug=False)
    # host pre-permuted layouts -> fully contiguous DMA descriptors
    # x: [p, lc, dc, l'] where d = dc*128+p, l = lc*512+l'
    x_d = nc.dram_tensor("x", [128, 4, 4, 512], BF16, kind="ExternalInput")
    # wT split in 4 sections s = 2*pair + (0 q | 1 k): [s, p, dc, o]
    wqk_d = nc.dram_tensor("wqkT", [4, 128, 4, 128], BF16, kind="ExternalInput")
    wv_d = nc.dram_tensor("wvT", [128, 4, 256], BF16, kind="ExternalInput")
    out_d = nc.dram_tensor("out", [256, L], F32, kind="ExternalOutput")

    NJB = L // 128  # 16 key blocks
    NIC = L // 512  # 4 query chunks

    with ExitStack() as ctx:
        tc = ctx.enter_context(tile.TileContext(nc))
        const = ctx.enter_context(tc.tile_pool(name="const", bufs=1))
        qkp = ctx.enter_context(tc.tile_pool(name="qkp", bufs=1))
        vtp = ctx.enter_context(tc.tile_pool(name="vtp", bufs=1))
        sx = ctx.enter_context(tc.tile_pool(name="sx", bufs=6))
        nrm = ctx.enter_context(tc.tile_pool(name="nrm", bufs=4))
        outp = ctx.enter_context(tc.tile_pool(name="outp", bufs=4))
        drp = ctx.enter_context(tc.tile_pool(name="drp", bufs=4, space="DRAM"))
        ps_st = ctx.enter_context(tc.tile_pool(name="ps_st", bufs=2, space="PSUM"))
        ps_o = ctx.enter_context(tc.tile_pool(name="ps_o", bufs=4, space="PSUM"))

        # ---- PE warmup + load inputs ----
        # a few matmuls on zeros keep the PE busy while the first input DMAs
        # land, so the HAM clock gate starts opening (1.2 -> 2.4 GHz).
        scr_sb = const.tile([128, 512], BF16, tag="scr")
        nc.vector.memset(scr_sb[:], 0.0)
        warm_ps = ps_st.tile([128, 1024], F32, tag="st", name="warm")
        for _ in range(8):
            nc.tensor.matmul(warm_ps[:, 0:512], scr_sb[:, 0:128], scr_sb[:])

        wqk_sb = const.tile([128, 4, 4, 128], BF16, tag="wqk")
        wv_sb = const.tile([128, 4, 256], BF16, tag="wv")
        x_sb = const.tile([128, 4, 4, 512], BF16, tag="x")
        # Each HWDGE queue sustains only ~200GB/s but drains FIFO, so: split
        # the critical first-projection set (wqk sections 0-1 + x chunk 0)
        # across BOTH rings so it lands in ~1.2us per ring, with the bulk
        # chunks queued behind it in deadline order. Config instructions also
        # serialize per-ring (~0.65us each) — another reason to use two.
        nc.sync.dma_start(out=wqk_sb[:, 0, :, :], in_=wqk_d[0])
        nc.scalar.dma_start(out=wqk_sb[:, 1, :, :], in_=wqk_d[1])
        nc.sync.dma_start(out=x_sb[:, 0, 0:2, :], in_=x_d[:, 0, 0:2, :])
        nc.scalar.dma_start(out=x_sb[:, 0, 2:4, :], in_=x_d[:, 0, 2:4, :])
        nc.sync.dma_start(out=x_sb[:, 1, :, :], in_=x_d[:, 1, :, :])
        nc.scalar.dma_start(out=wv_sb[:], in_=wv_d[:])
        nc.sync.dma_start(out=x_sb[:, 3, :, :], in_=x_d[:, 3, :, :])
        nc.scalar.dma_start(out=x_sb[:, 2, :, :], in_=x_d[:, 2, :, :])
        nc.scalar.dma_start(out=wqk_sb[:, 2, :, :], in_=wqk_d[2])
        nc.scalar.dma_start(out=wqk_sb[:, 3, :, :], in_=wqk_d[3])
        ones_sb = const.tile([1, 64], F32, tag="ones")
        nc.vector.memset(ones_sb[:], 1.0)

        q_sb = [qkp.tile([128, L], BF16, tag=f"q{p}", name=f"q{p}") for p in range(2)]
        k_sb = [qkp.tile([128, L], BF16, tag=f"k{p}", name=f"k{p}") for p in range(2)]
        vt_sb = [vtp.tile([128, 4, 65], BF16, tag=f"vt{jb}", name=f"vt{jb}") for jb in range(NJB)]

        # Projection groups run in 1-bank [128,512] PSUM tiles from the shared
        # "o" pool so they never contend with the exp-feeding st pipeline.
        def g_qk(p, qk, lc):
            # one 512-wide column group of the Q (qk=0) or K (qk=1)
            # projection for head-pair p
            def f():
                dst = q_sb[p] if qk == 0 else k_sb[p]
                ps = ps_o.tile([128, 512], F32, tag="o", name="projg")
                for dc in range(4):
                    nc.tensor.matmul(
                        ps[:],
                        wqk_sb[:, 2 * p + qk, dc, :],
                        x_sb[:, lc, dc, :],
                        start=(dc == 0),
                        stop=(dc == 3),
                    )
                nc.vector.tensor_copy(dst[:, lc * 512 : (lc + 1) * 512], ps[:])

            return f

        def g_qk_split(p, qk, lc, dcs):
            # g_qk sub-split into per-iteration pieces (each <=2 matmuls,
            # ~426ns) so filler load never spikes a single exp interval.
            # Returns a list of callables; schedule them on consecutive jbs.
            state = {}

            def part(dc_list):
                def f():
                    if 0 in dc_list:
                        state["ps"] = ps_o.tile(
                            [128, 512], F32, tag="o", name="projg"
                        )
                    for dc in dc_list:
                        nc.tensor.matmul(
                            state["ps"][:],
                            wqk_sb[:, 2 * p + qk, dc, :],
                            x_sb[:, lc, dc, :],
                            start=(dc == 0),
                            stop=(dc == 3),
                        )
                    if 3 in dc_list:
                        dst = q_sb[p] if qk == 0 else k_sb[p]
                        nc.vector.tensor_copy(
                            dst[:, lc * 512 : (lc + 1) * 512], state["ps"][:]
                        )

                return f

            return [part(dcl) for dcl in dcs]

        def g_vt(jb):
            def f():
                nc.vector.memset(vt_sb[jb][:, :, 64:65], 1.0)
                ps = ps_o.tile([128, 512], F32, tag="o", name="projv")
                for dc in range(4):
                    nc.tensor.matmul(
                        ps[:, 0:256],
                        x_sb[:, jb // 4, dc, (jb % 4) * 128 : (jb % 4 + 1) * 128],
                        wv_sb[:, dc, :],
                        start=(dc == 0),
                        stop=(dc == 3),
                    )
                nc.vector.tensor_copy(
                    vt_sb[jb][:, :, 0:64],
                    ps[:, 0:256].rearrange("par (h e) -> par h e", e=64),
                )

            return f

        def attn_block(p, ic, fillers=(), last=False):
            # scores+softmax+attn@v for head pair p, query chunk ic (512 wide)
            # fillers: {jb: [callables]} — projection groups interleaved into
            # the loop to fill PE slack without starving ScalarE
            # last: tail-optimized normalization (ScalarE den copies, GpSimd
            # mul, parallel out-DMA queues, PE broadcast)
            fillers = dict(fillers)
            i0 = ic * 512

            def st_mms(jb):
                # St[j, i] for both heads of the pair, row-packed in the PE
                st = ps_st.tile([128, 1024], F32, tag="st")
                for hp in range(2):
                    nc.tensor.matmul(
                        st[:, hp * 512 : (hp + 1) * 512],
                        k_sb[p][hp * 64 : (hp + 1) * 64, jb * 128 : (jb + 1) * 128],
                        q_sb[p][hp * 64 : (hp + 1) * 64, i0 : i0 + 512],
                        start=True,
                        stop=True,
                    )
                return st

            o_ps = [ps_o.tile([65, 512], F32, tag="o", name="o_acc") for _ in range(2)]
            st_cur = st_mms(0)
            for jb in range(NJB):
                se = sx.tile([128, 1024], BF16, tag="se")
                nc.scalar.activation(se[:], st_cur[:], Exp, scale=SCALE)
                if jb + 1 < NJB:
                    st_cur = st_mms(jb + 1)
                for f in fillers.get(jb, ()):
                    f()
                for hp in range(2):
                    nc.tensor.matmul(
                        o_ps[hp][:],
                        vt_sb[jb][:, 2 * p + hp, :],
                        se[:, hp * 512 : (hp + 1) * 512],
                        start=(jb == 0),
                        stop=(jb == NJB - 1),
                    )

            if not last:
                # normalize and write out. The o_ps ring slots gate the NEXT
                # block's projection fillers, so the PSUM accumulator must be
                # released ASAP: ONE [65,512] DVE copy (same cost as copying
                # just the den row — DVE time is free-dim-bound) moves o+den
                # to SBUF, then the whole chain (1/den on DVE, broadcast via
                # DRAM bounce, multiply) runs off SBUF without touching PSUM.
                for hp in range(2):
                    hh = 2 * p + hp
                    den_sb = nrm.tile([1, 512], F32, tag="den")
                    nc.vector.tensor_copy(den_sb[:], o_ps[hp][64:65, :])
                    oc = nrm.tile([64, 512], F32, tag="oc")
                    nc.vector.tensor_copy(oc[:], o_ps[hp][0:64, :])
                    recip = nrm.tile([1, 512], F32, tag="recip")
                    # NB: approx-recip reads garbage from PSUM on HW; SBUF in
                    # only, and DVE lanes are partition-aligned: in/out must
                    # start at the same partition (hence the den copy to p0).
                    nc.vector.reciprocal_approx_fast(out=recip[:], in_=den_sb[:])
                    rbc = nrm.tile([64, 512], F32, tag="rbc")
                    dbounce = drp.tile([1, 512], F32, tag="db", name="db")
                    nc.sync.dma_start(out=dbounce[:], in_=recip[:])
                    nc.sync.dma_start(
                        out=rbc[:],
                        in_=bass.AP(
                            tensor=dbounce.tensor,
                            offset=dbounce.offset,
                            ap=[[0, 64], [1, 512]],
                        ),
                    )
                    ot = outp.tile([64, 512], F32, tag="ot")
                    nc.vector.tensor_mul(ot[:], oc[0:64, :], rbc[:])
                    nc.sync.dma_start(
                        out=out_d[hh * 64 : (hh + 1) * 64, i0 : i0 + 512], in_=ot[:]
                    )
                return

            # tail-optimized final normalization. Hop count is what matters:
            # den-copy + reciprocal back-to-back on DVE (no cross-engine sem
            # between them), broadcast matmul on a pre-warmed PE (six dummy
            # matmuls run during the last exp so the HAM clock stays up),
            # broadcast read-back on the idle ScalarE, muls on DVE, and the
            # two out-DMAs on separate queues.
            den_sb = [nrm.tile([1, 512], F32, tag="den", name=f"den{hp}") for hp in range(2)]
            oc = [nrm.tile([64, 512], F32, tag="oc", name=f"oc{hp}") for hp in range(2)]
            recip = [nrm.tile([1, 512], F32, tag="recip", name=f"recip{hp}") for hp in range(2)]
            rbc = [nrm.tile([64, 512], F32, tag="rbc", name=f"rbc{hp}") for hp in range(2)]
            bc_ps = [ps_st.tile([128, 1024], F32, tag="st", name=f"bc{hp}") for hp in range(2)]
            ot = [outp.tile([64, 512], F32, tag="ot", name=f"ot{hp}") for hp in range(2)]
            for _ in range(4):
                nc.tensor.matmul(
                    bc_ps[0][64:128, 0:512], scr_sb[:, 0:64], scr_sb[:],
                    start=True, stop=True,
                )
            for hp in range(2):
                nc.vector.tensor_copy(den_sb[hp][:], o_ps[hp][64:65, :])
                nc.vector.reciprocal_approx_fast(out=recip[hp][:], in_=den_sb[hp][:])
                nc.scalar.copy(oc[hp][:], o_ps[hp][0:64, :])
            for _ in range(3):
                nc.tensor.matmul(
                    bc_ps[0][64:128, 0:512], scr_sb[:, 0:64], scr_sb[:],
                    start=True, stop=True,
                )
            for hp in range(2):
                nc.tensor.matmul(
                    bc_ps[hp][0:64, 0:512], ones_sb[:], recip[hp][:],
                    start=True, stop=True,
                )
                nc.scalar.copy(rbc[hp][:], bc_ps[hp][0:64, 0:512])
            # muls are all-SBUF (oc x rbc), so GpSimd can take one in parallel
            nc.gpsimd.tensor_mul(ot[0][:], oc[0][:], rbc[0][:])
            nc.vector.tensor_mul(ot[1][:], oc[1][:], rbc[1][:])
            hh0 = 2 * p
            nc.sync.dma_start(
                out=out_d[hh0 * 64 : (hh0 + 1) * 64, i0 : i0 + 512], in_=ot[0][:]
            )
            nc.scalar.dma_start(
                out=out_d[(hh0 + 1) * 64 : (hh0 + 2) * 64, i0 : i0 + 512], in_=ot[1][:]
            )

        # prologue: just enough projection for the first scores (q0/k0 column
        # group 0); everything else is interleaved as fillers.
        # Constraints: vt[j] before av(j) of block (0,0); k[p] group m fully
        # projected+cast before st(4m) is EMITTED (iter 4m-1) of the first
        # block using pair p; q[p] group lc before block (p,lc). Projection
        # groups are sub-split so no single exp interval carries more than
        # ~0.43us of filler on top of st+av(+vt).
        def sched(*entries):
            m = {}
            for start, parts in entries:
                for i, part in enumerate(parts):
                    m.setdefault(start + i, []).append(part)
            return m

        SPLIT_211 = [[0, 1], [2], [3]]
        SPLIT_1111 = [[0], [1], [2], [3]]

        g_qk(0, 0, 0)()
        g_qk(0, 1, 0)()
        attn_block(0, 0, sched(
            *[(jb, [g_vt(jb)]) for jb in range(NJB)],
            (0, g_qk_split(0, 1, 1, SPLIT_211)),
            (3, g_qk_split(0, 1, 2, SPLIT_211)),
            (7, g_qk_split(0, 1, 3, SPLIT_1111)),
            (11, g_qk_split(0, 0, 1, SPLIT_1111)),
        ))
        attn_block(0, 1, sched((2, g_qk_split(0, 0, 2, SPLIT_1111))))
        attn_block(0, 2, sched((2, g_qk_split(0, 0, 3, SPLIT_1111))))
        attn_block(0, 3, sched(
            (1, g_qk_split(1, 1, 0, SPLIT_1111)),
            (8, g_qk_split(1, 0, 0, SPLIT_1111)),
        ))
        attn_block(1, 0, sched(
            (0, g_qk_split(1, 1, 1, SPLIT_211)),
            (3, g_qk_split(1, 1, 2, SPLIT_1111)),
            (7, g_qk_split(1, 1, 3, SPLIT_1111)),
            (11, g_qk_split(1, 0, 1, SPLIT_1111)),
        ))
        attn_block(1, 1, sched((2, g_qk_split(1, 0, 2, SPLIT_1111))))
        attn_block(1, 2, sched((2, g_qk_split(1, 0, 3, SPLIT_1111))))
        attn_block(1, 3, last=True)

    nc.finalize()
    return nc


def _get_nc():
    if "nc" not in _COMPILED:
        _COMPILED["nc"] = _build_nc()
    return _COMPILED["nc"]


def _prep_inputs(x, w_qkv):
    """Per-core input maps (host-side sharding)."""
    import ml_dtypes

    bf16 = ml_dtypes.bfloat16
    in_maps = []
    for c in range(N_CORES):
        b, g = c // 2, c % 2
        # x[b] [512, 2048] -> [p, lc, dc, l'] so every DMA descriptor is a
        # 4KB contiguous run
        xb = np.ascontiguousarray(
            x[b].reshape(4, 128, 4, 512).transpose(1, 2, 0, 3)
        ).astype(bf16)
        # w rows for this head group, transposed then laid out in 4 sections
        # s = 2*pair + (0 q | 1 k): [s, p, dc, o]; v separate — all DMAs are
        # fully contiguous
        wq_rows = w_qkv[256 * g : 256 * (g + 1), :]  # [256, 512]
        wk_rows = w_qkv[512 + 256 * g : 512 + 256 * (g + 1), :]
        wv_rows = w_qkv[1024 + 256 * g : 1024 + 256 * (g + 1), :]
        # section s covers head pair p = s//2, q (s%2==0) or k: 128 columns of
        # wT = rows 128*p..128*(p+1) of the q|k block
        secs = []
        for p in range(2):
            for rows in (wq_rows, wk_rows):
                sec = rows[128 * p : 128 * (p + 1), :].T  # [512(d), 128(o)]
                secs.append(sec.reshape(4, 128, 128).transpose(1, 0, 2))
        wqkT = np.ascontiguousarray(np.stack(secs, axis=0)).astype(bf16)
        wvT = np.ascontiguousarray(
            wv_rows.T.reshape(4, 128, 256).transpose(1, 0, 2)
        ).astype(bf16)
        in_maps.append({"x": xb, "wqkT": wqkT, "wvT": wvT})
    return in_maps


def kernel(x, w_qkv):
    global LAST_RESULTS
    from concourse.bass_utils import run_bass_kernel_spmd

    nc = _get_nc()
    in_maps = _prep_inputs(np.asarray(x), np.asarray(w_qkv))
    res = run_bass_kernel_spmd(
        nc, in_maps, core_ids=list(range(N_CORES)), trace=TRACE
    )
    LAST_RESULTS = res
    out = np.empty((B, D, L), dtype=np.float32)
    for c in range(N_CORES):
        b, g = c // 2, c % 2
        out[b, 256 * g : 256 * (g + 1), :] = res.results[c]["out"]
    return out
